# revision 2
# baseline (speedup 1.0000x reference)
"""DIN Trainium2 kernel: 8-core data-parallel over batch.

kernel(**inputs) -> [B, 1] float32. Self-contained (imports concourse only).
"""
import os
from contextlib import ExitStack

import numpy as np

N_CORES = 8
BATCH = 4096
VOCAB = 100000
BLK = 128

_cache = {}


def _build():
    import concourse.bacc as bacc
    import concourse.tile as tile
    import din_kernel as dk

    b_local = BATCH // N_CORES
    nc = bacc.Bacc("TRN2", target_bir_lowering=False, debug=False,
                   enable_asserts=False, num_devices=N_CORES)
    t = dk.declare_tensors(nc, b_local=b_local, V=VOCAB)
    with tile.TileContext(nc) as tc:
        with ExitStack() as ctx:
            dk.build(ctx, tc, t, b_local=b_local, blk=BLK)
    nc.compile()
    return nc


def _install_ntff_hook():
    """Fabricate antenv.axon_hooks so trace=True works under axon."""
    import sys, types
    if "antenv.axon_hooks" in sys.modules:
        return
    mod = types.ModuleType("antenv.axon_hooks")
    _h = [None]
    mod.set_axon_ntff_profile_hook = lambda h: _h.__setitem__(0, h)
    mod.get_axon_ntff_profile_hook = lambda: _h[0]
    sys.modules["antenv.axon_hooks"] = mod
    import antenv
    antenv.axon_hooks = mod
    try:
        from trn_agent_boot.trn_boot import _ntff_profile_via_ctypes
        mod.set_axon_ntff_profile_hook(
            _ntff_profile_via_ctypes("/opt/axon/libaxon_pjrt.so"))
    except Exception:
        pass


def kernel(**inputs):
    import din_kernel as dk
    from concourse import bass_utils

    if bool(int(os.environ.get("DIN_TRACE", "0"))):
        _install_ntff_hook()
    if "nc" not in _cache:
        _cache["nc"] = _build()
    nc = _cache["nc"]

    in_maps = dk.host_prep(inputs, N_CORES)
    res = bass_utils.run_bass_kernel_spmd(
        nc, in_maps, core_ids=list(range(N_CORES)),
        trace=bool(int(os.environ.get("DIN_TRACE", "0"))))
    if res.exec_time_ns is not None:
        print(f"HW exec time: {res.exec_time_ns} ns")
    outs = res.results
    full = np.concatenate([outs[c]["out"] for c in range(N_CORES)], axis=0)
    return full.astype(np.float32)
